# revision 4
# baseline (speedup 1.0000x reference)
"""Multi-head attention (no softmax) on 8 trn2 NeuronCores.

Reference: out = ((x @ Wqkv.T -> q,k,v per head) ; (q @ k.T * s) @ v ; concat ; @ Wproj.T)

Because there is no softmax the attention is linear:
    (q @ k.T) @ v == q @ (k.T @ v),  k.T @ v is only 64x64 per head,
so the T x T score matrices never need to exist. Per head:
    M_h = (s * k_h).T @ v_h        (64 x 64, reduced over ALL tokens of the batch)
    out += (q_h @ M_h) @ Wproj_h.T

Sharding: token-parallel. Core c owns batch b=c//2, token half c%2 (512 tokens).
M_h needs a reduction over the full batch -> tiny (128,512)=256KB AllReduce
between the two cores of each batch, overlapped with the q matmuls.

All matmuls run in float32r (full PE rate for free dim >= 256; fp32 would be
4x slower). Inputs are pre-rounded to fp32r on the host (exact-on-hardware),
intermediates are rounded by the PSUM->SBUF eviction copies. The head-dim
scale 1/8 is folded into W_k on the host (exact, power of two).

Weights are fed pre-transposed/pre-permuted so every matmul operand has the
contraction dim on partitions with unit-stride DMAs:
  wqkvT (E, 3E): cols 0:E = q features grouped h*64+j, E:2E = k (scaled), 2E:3E = v
  wpT   (E, E):  wpT[f, o] = W_proj[o, f]
  xT_c  (E, 512) per core.
"""

import numpy as np

B, T, E = 4, 1024, 1024
NH, HD = 16, 64
N_CORES = 8
TPC = T // 2  # tokens per core = 512

_built = None


def _round_fp32r(a: np.ndarray) -> np.ndarray:
    """Round fp32 to fp32r (11 explicit mantissa bits, RNE) — matches HW."""
    u = np.ascontiguousarray(a, dtype=np.float32).view(np.uint32).astype(np.uint64)
    u = u + 0x7FF + ((u >> 12) & 1)
    u = (u & ~np.uint64(0xFFF)).astype(np.uint32)
    return u.view(np.float32).reshape(a.shape)


def _build():
    """Build + compile the 8-core SPMD Bass program once."""
    global _built
    if _built is not None:
        return _built

    import concourse.mybir as mybir
    import concourse.tile as tile
    from concourse import bacc

    f32 = mybir.dt.float32
    f32r = mybir.dt.float32r

    nc = bacc.Bacc("TRN2", target_bir_lowering=False, debug=False, num_devices=N_CORES)
    xT = nc.dram_tensor("xT", [E, TPC], f32r, kind="ExternalInput").ap()
    wqkvT = nc.dram_tensor("wqkvT", [E, 3 * E], f32r, kind="ExternalInput").ap()
    wpT = nc.dram_tensor("wpT", [E, E], f32r, kind="ExternalInput").ap()
    out = nc.dram_tensor("out", [TPC, E], f32, kind="ExternalOutput").ap()

    def evict(i, dst, src):
        # spread PSUM->SBUF eviction copies across DVE and ACT
        if i % 2 == 0:
            nc.vector.tensor_copy(dst, src)
        else:
            nc.scalar.copy(dst, src)

    with tile.TileContext(nc) as tc:
        with (
            tc.tile_pool(name="xp", bufs=1) as xp,
            tc.tile_pool(name="mres", bufs=1) as mres,
            tc.tile_pool(name="dram", bufs=1, space="DRAM") as dram,
            tc.tile_pool(name="psA", bufs=4, space="PSUM") as psA,
            tc.tile_pool(name="psM", bufs=2, space="PSUM") as psM,
        ):
            xsb = []
            for e in range(8):
                t = xp.tile([128, TPC], f32r, tag=f"x{e}")
                nc.sync.dma_start(t[:], xT[128 * e:128 * (e + 1), :])
                xsb.append(t)

            # ---- phase 1: kv = x @ Wkv.T  (token-major, (512t, 2048f)) ----
            with (
                tc.tile_pool(name="wkvp", bufs=1) as wkvp,
                tc.tile_pool(name="kvp", bufs=1) as kvp,
            ):
                wkv = []
                for e in range(8):
                    t = wkvp.tile([128, 2 * E], f32r, tag=f"wkv{e}")
                    nc.sync.dma_start(t[:], wqkvT[128 * e:128 * (e + 1), E:3 * E])
                    wkv.append(t)
                kvsb = [kvp.tile([128, 2 * E], f32r, tag=f"kv{tt}", name=f"kv{tt}") for tt in range(4)]
                i = 0
                for tt in range(4):
                    for fc in range(4):
                        ps = psA.tile([128, 512], f32, tag="big")
                        for e in range(8):
                            nc.tensor.matmul(
                                ps[:],
                                xsb[e][:, 128 * tt:128 * (tt + 1)],
                                wkv[e][:, 512 * fc:512 * (fc + 1)],
                                start=(e == 0), stop=(e == 7),
                            )
                        evict(i, kvsb[tt][:, 512 * fc:512 * (fc + 1)], ps[:])
                        i += 1

                # ---- phase 2: per-head-pair M blocks (compact (128, 512)) ----
                # Mfull_blk = kv_k_blk.T @ kv_v_blk (128x128); keep only the two
                # diagonal 64x64 sub-blocks (cross-head terms are not wanted).
                Msb = mres.tile([128, 512], f32, tag="Msb")
                for g in range(2):
                    mp = psM.tile([128, 512], f32, tag="mp")
                    for j in range(4):
                        blk = 4 * g + j
                        for tt in range(4):
                            nc.tensor.matmul(
                                mp[:, 128 * j:128 * (j + 1)],
                                kvsb[tt][:, 128 * blk:128 * (blk + 1)],
                                kvsb[tt][:, E + 128 * blk:E + 128 * (blk + 1)],
                                start=(tt == 0), stop=(tt == 3),
                            )
                    for j in range(4):
                        blk = 4 * g + j
                        evict(j, Msb[0:64, 64 * blk:64 * blk + 64],
                              mp[0:64, 128 * j:128 * j + 64])
                        evict(j + 1, Msb[64:128, 64 * blk:64 * blk + 64],
                              mp[64:128, 128 * j + 64:128 * (j + 1)])

            # ---- phase 3: AllReduce M across the batch's core pair ----
            bin_ = dram.tile([128, 512], f32)
            bout = dram.tile([128, 512], f32)
            nc.sync.dma_start(bin_[:], Msb[:])
            nc.gpsimd.collective_compute(
                "AllReduce",
                mybir.AluOpType.add,
                replica_groups=[[0, 1], [2, 3], [4, 5], [6, 7]],
                ins=[bin_.opt()],
                outs=[bout.opt()],
            )
            Mr = mres.tile([128, 512], f32, tag="Mr")
            nc.sync.dma_start(Mr[:], bout[:])
            # scatter to block-diagonal (128, 1024) fp32r for the att matmuls
            Mbd = mres.tile([128, 1024], f32r, tag="Mbd")
            nc.vector.memset(Mbd[:].bitcast(f32), 0.0)
            for blk in range(8):
                nc.vector.tensor_copy(Mbd[0:64, 128 * blk:128 * blk + 64],
                                      Mr[0:64, 64 * blk:64 * blk + 64])
                nc.vector.tensor_copy(Mbd[64:128, 128 * blk + 64:128 * (blk + 1)],
                                      Mr[64:128, 64 * blk:64 * blk + 64])

            # ---- phase 4/5/6: q, att, out ----
            with tc.tile_pool(name="qp", bufs=1) as qp:
                qsb = [qp.tile([128, TPC], f32r, tag=f"q{f}", name=f"q{f}") for f in range(8)]
                attsb = [qp.tile([128, TPC], f32r, tag=f"att{f}", name=f"att{f}") for f in range(8)]
                with tc.tile_pool(name="wqp", bufs=1) as wqp:
                    wq = []
                    for e in range(8):
                        t = wqp.tile([128, E], f32r, tag=f"wq{e}")
                        nc.sync.dma_start(t[:], wqkvT[128 * e:128 * (e + 1), 0:E])
                        wq.append(t)
                    # qT (feature-major, (1024f, 512t))
                    for fq in range(8):
                        ps = psA.tile([128, 512], f32, tag="big")
                        for e in range(8):
                            nc.tensor.matmul(
                                ps[:],
                                wq[e][:, 128 * fq:128 * (fq + 1)],
                                xsb[e][:],
                                start=(e == 0), stop=(e == 7),
                            )
                        evict(fq, qsb[fq][:], ps[:])

                # attT_blk = Mbd_blk.T @ qT_blk  (block-diagonal M)
                for blk in range(8):
                    ps = psA.tile([128, 512], f32, tag="big")
                    nc.tensor.matmul(ps[:], Mbd[:, 128 * blk:128 * (blk + 1)],
                                     qsb[blk][:], start=True, stop=True)
                    evict(blk, attsb[blk][:], ps[:])

                # out = attT.T @ wpT  ((512t, 1024o))
                with (
                    tc.tile_pool(name="wpp", bufs=1) as wpp,
                    tc.tile_pool(name="op", bufs=3) as op,
                ):
                    wp = []
                    for f in range(8):
                        t = wpp.tile([128, E], f32r, tag=f"wp{f}")
                        nc.sync.dma_start(t[:], wpT[128 * f:128 * (f + 1), :])
                        wp.append(t)
                    i = 0
                    for tt in range(4):
                        for oc in range(2):
                            ps = psA.tile([128, 512], f32, tag="big")
                            for f in range(8):
                                nc.tensor.matmul(
                                    ps[:],
                                    attsb[f][:, 128 * tt:128 * (tt + 1)],
                                    wp[f][:, 512 * oc:512 * (oc + 1)],
                                    start=(f == 0), stop=(f == 7),
                                )
                            ot = op.tile([128, 512], f32, tag="osb")
                            evict(i, ot[:], ps[:])
                            i += 1
                            nc.sync.dma_start(
                                out[128 * tt:128 * (tt + 1), 512 * oc:512 * (oc + 1)],
                                ot[:],
                            )

    nc.compile()
    _built = nc
    return nc


LAST_RESULTS = None  # BassKernelResults of the most recent kernel() call


def kernel(x: np.ndarray, W_qkv: np.ndarray, W_proj: np.ndarray) -> np.ndarray:
    global LAST_RESULTS
    from concourse import bass_utils

    nc = _build()

    x = np.ascontiguousarray(x, dtype=np.float32)
    W_qkv = np.ascontiguousarray(W_qkv, dtype=np.float32)
    W_proj = np.ascontiguousarray(W_proj, dtype=np.float32)

    # head-grouping permutation: grouped feature h*64+j <- original row j*16+h
    perm = np.arange(E).reshape(HD, NH).T.ravel()
    Wq_g = W_qkv[perm]
    Wk_g = W_qkv[E + perm] * np.float32(HD ** -0.5)  # exact: 1/8
    Wv_g = W_qkv[2 * E + perm]
    wqkvT_np = _round_fp32r(np.concatenate([Wq_g, Wk_g, Wv_g], 0).T)
    wpT_np = _round_fp32r(W_proj.T)

    in_maps = []
    for c in range(N_CORES):
        b, half = c // 2, c % 2
        xT_c = _round_fp32r(x[b, half * TPC:(half + 1) * TPC, :].T)
        in_maps.append({"xT": xT_c, "wqkvT": wqkvT_np, "wpT": wpT_np})

    res = bass_utils.run_bass_kernel_spmd(nc, in_maps, core_ids=list(range(N_CORES)))
    LAST_RESULTS = res

    out = np.empty((B, T, E), dtype=np.float32)
    for c in range(N_CORES):
        b, half = c // 2, c % 2
        out[b, half * TPC:(half + 1) * TPC, :] = res.results[c]["out"]
    return out


# revision 7
# speedup vs baseline: 1.1580x; 1.1580x over previous
"""Multi-head attention (no softmax) on 8 trn2 NeuronCores.

Reference: out = ((x @ Wqkv.T -> q,k,v per head) ; (q @ k.T * s) @ v ; concat ; @ Wproj.T)

Because there is no softmax the attention is linear:
    (q @ k.T) @ v == q @ (k.T @ v),  k.T @ v is only 64x64 per head,
so the T x T score matrices never need to exist. Per head:
    M_h = (s * k_h).T @ v_h        (64 x 64, reduced over ALL tokens of the batch)
    out += (q_h @ M_h) @ Wproj_h.T

Sharding: token-parallel. Core c owns batch b=c//2, token half c%2 (512 tokens).
M_h needs a reduction over the full batch -> tiny 256KB AllGather between the
two cores of each batch (+ local add), overlapped with the q matmuls.

All matmuls run in float32r (full PE rate; fp32 is 4x slower). Inputs are
pre-rounded to fp32r on the host (matmul is then exact), intermediates are
rounded by the PSUM->SBUF eviction copies. The head-dim scale 1/8 is folded
into W_k on the host (exact, power of two).

Weights are fed pre-transposed/pre-permuted so every matmul operand has the
contraction dim on partitions with unit-stride DMAs:
  wqkvT (E, 3E): cols 0:E = q features grouped h*64+j, E:2E = k (scaled), 2E:3E = v
  wpT   (E, E):  wpT[f, o] = W_proj[o, f]
  xT_c  (E, 512) per core.

DMA triggers are split between the Sync queue (x, weights, outputs - issued in
program order, which paces the phases) and the GpSimd queue (collective bounce
+ gather loads - so the collective fires the moment its input is ready instead
of sitting behind weight loads in the Sync queue).
"""

import numpy as np

B, T, E = 4, 1024, 1024
NH, HD = 16, 64
N_CORES = 8
TPC = T // 2  # tokens per core = 512

_built = None


def _round_fp32r(a: np.ndarray) -> np.ndarray:
    """Round fp32 to fp32r (11 explicit mantissa bits, RNE) — matches HW."""
    u = np.ascontiguousarray(a, dtype=np.float32).view(np.uint32).astype(np.uint64)
    u = u + 0x7FF + ((u >> 12) & 1)
    u = (u & ~np.uint64(0xFFF)).astype(np.uint32)
    return u.view(np.float32).reshape(a.shape)


def _build():
    """Build + compile the 8-core SPMD Bass program once."""
    global _built
    if _built is not None:
        return _built

    import concourse.mybir as mybir
    import concourse.tile as tile
    from concourse import bacc

    f32 = mybir.dt.float32
    f32r = mybir.dt.float32r

    nc = bacc.Bacc("TRN2", target_bir_lowering=False, debug=False, num_devices=N_CORES)
    xT = nc.dram_tensor("xT", [E, TPC], f32r, kind="ExternalInput").ap()
    wqkvT = nc.dram_tensor("wqkvT", [E, 3 * E], f32r, kind="ExternalInput").ap()
    wpT = nc.dram_tensor("wpT", [E, E], f32r, kind="ExternalInput").ap()
    out = nc.dram_tensor("out", [TPC, E], f32, kind="ExternalOutput").ap()

    def evict(i, dst, src):
        # spread PSUM->SBUF eviction copies across DVE and ACT
        if i % 2 == 0:
            nc.vector.tensor_copy(dst, src)
        else:
            nc.scalar.copy(dst, src)

    with tile.TileContext(nc) as tc:
        with (
            tc.tile_pool(name="xp", bufs=1) as xp,
            tc.tile_pool(name="wkvp", bufs=2) as wkvp,
            tc.tile_pool(name="kvp", bufs=1) as kvp,
            tc.tile_pool(name="wqp", bufs=1) as wqp,
            tc.tile_pool(name="wpp", bufs=1) as wpp,
            tc.tile_pool(name="qp", bufs=1) as qp,
            tc.tile_pool(name="mres", bufs=1) as mres,
            tc.tile_pool(name="op", bufs=3) as op,
            tc.tile_pool(name="dram", bufs=1, space="DRAM") as dram,
            tc.tile_pool(name="psA", bufs=4, space="PSUM") as psA,
            tc.tile_pool(name="psM", bufs=2, space="PSUM") as psM,
        ):
            # ---- input DMAs on the Sync queue, in the order phases need them
            xsb = []
            for e in range(8):
                t = xp.tile([128, TPC], f32r, tag=f"x{e}")
                nc.sync.dma_start(t[:], xT[128 * e:128 * (e + 1), :])
                xsb.append(t)

            # wkv streamed in four 512-column groups (k cols, then v cols)
            def load_wkv(fc):
                tiles = []
                for e in range(8):
                    t = wkvp.tile([128, 512], f32r, tag=f"wkv{e}", name=f"wkv{fc}_{e}")
                    nc.sync.dma_start(
                        t[:], wqkvT[128 * e:128 * (e + 1),
                                    E + 512 * fc:E + 512 * (fc + 1)])
                    tiles.append(t)
                return tiles

            wkv_groups = [load_wkv(fc) for fc in range(4)]

            wq = []
            for e in range(8):
                t = wqp.tile([128, E], f32r, tag=f"wq{e}")
                nc.sync.dma_start(t[:], wqkvT[128 * e:128 * (e + 1), 0:E])
                wq.append(t)
            wp = []
            for f in range(8):
                t = wpp.tile([128, E], f32r, tag=f"wp{f}")
                nc.sync.dma_start(t[:], wpT[128 * f:128 * (f + 1), :])
                wp.append(t)

            # ---- phase 1: kv = x @ Wkv.T  (token-major, (512t, 2048f)) ----
            kvsb = [kvp.tile([128, 2 * E], f32r, tag=f"kv{tt}", name=f"kv{tt}")
                    for tt in range(4)]
            i = 0
            for fc in range(4):
                wkv = wkv_groups[fc]
                for tt in range(4):
                    ps = psA.tile([128, 512], f32, tag="big")
                    for e in range(8):
                        nc.tensor.matmul(
                            ps[:],
                            xsb[e][:, 128 * tt:128 * (tt + 1)],
                            wkv[e][:],
                            start=(e == 0), stop=(e == 7),
                        )
                    evict(i, kvsb[tt][:, 512 * fc:512 * (fc + 1)], ps[:])
                    i += 1

            # ---- phase 2: per-head-pair M blocks (compact (128, 512)) ----
            # Mfull_blk = kv_k_blk.T @ kv_v_blk (128x128); keep only the two
            # diagonal 64x64 sub-blocks (cross-head terms are not wanted).
            Msb = mres.tile([128, 512], f32, tag="Msb")
            for g in range(2):
                mp = psM.tile([128, 512], f32, tag="mp")
                for j in range(4):
                    blk = 4 * g + j
                    for tt in range(4):
                        nc.tensor.matmul(
                            mp[:, 128 * j:128 * (j + 1)],
                            kvsb[tt][:, 128 * blk:128 * (blk + 1)],
                            kvsb[tt][:, E + 128 * blk:E + 128 * (blk + 1)],
                            start=(tt == 0), stop=(tt == 3),
                        )
                for j in range(4):
                    blk = 4 * g + j
                    evict(j, Msb[0:64, 64 * blk:64 * blk + 64],
                          mp[0:64, 128 * j:128 * j + 64])
                    evict(j + 1, Msb[64:128, 64 * blk:64 * blk + 64],
                          mp[64:128, 128 * j + 64:128 * (j + 1)])

            # ---- phase 3: AllGather partial M within the batch pair, add ----
            # (AllGather + local add is ~2x lower latency than AllReduce.)
            bin_ = dram.tile([128, 512], f32)
            bout = dram.tile([256, 512], f32)
            nc.gpsimd.dma_start(bin_[:], Msb[:])
            nc.gpsimd.collective_compute(
                "AllGather",
                mybir.AluOpType.bypass,
                replica_groups=[[0, 1], [2, 3], [4, 5], [6, 7]],
                ins=[bin_.opt()],
                outs=[bout.opt()],
            )
            MrA = mres.tile([128, 512], f32, tag="MrA")
            MrB = mres.tile([128, 512], f32, tag="MrB")
            nc.gpsimd.dma_start(MrA[:], bout[0:128, :])
            nc.gpsimd.dma_start(MrB[:], bout[128:256, :])
            Mr = mres.tile([128, 512], f32, tag="Mr")
            nc.vector.tensor_add(Mr[:], MrA[:], MrB[:])
            # scatter to block-diagonal (128, 1024) fp32r for the att matmuls
            Mbd = mres.tile([128, 1024], f32r, tag="Mbd")
            nc.vector.memset(Mbd[:].bitcast(f32), 0.0)
            for blk in range(8):
                nc.vector.tensor_copy(Mbd[0:64, 128 * blk:128 * blk + 64],
                                      Mr[0:64, 64 * blk:64 * blk + 64])
                nc.vector.tensor_copy(Mbd[64:128, 128 * blk + 64:128 * (blk + 1)],
                                      Mr[64:128, 64 * blk:64 * blk + 64])

            # ---- phase 4: qT (feature-major, (1024f, 512t)) ----
            qsb = [qp.tile([128, TPC], f32r, tag=f"q{f}", name=f"q{f}")
                   for f in range(8)]
            for fq in range(8):
                ps = psA.tile([128, 512], f32, tag="big")
                for e in range(8):
                    nc.tensor.matmul(
                        ps[:],
                        wq[e][:, 128 * fq:128 * (fq + 1)],
                        xsb[e][:],
                        start=(e == 0), stop=(e == 7),
                    )
                evict(fq, qsb[fq][:], ps[:])

            # ---- phase 5: attT_blk = Mbd_blk.T @ qT_blk ----
            # (writes back into the q tiles: q_blk is dead once its att is done)
            for blk in range(8):
                ps = psA.tile([128, 512], f32, tag="big")
                nc.tensor.matmul(ps[:], Mbd[:, 128 * blk:128 * (blk + 1)],
                                 qsb[blk][:], start=True, stop=True)
                evict(blk, qsb[blk][:], ps[:])
            attsb = qsb

            # ---- phase 6: out = attT.T @ wpT  ((512t, 1024o)) ----
            i = 0
            for tt in range(4):
                for oc in range(2):
                    ps = psA.tile([128, 512], f32, tag="big")
                    for f in range(8):
                        nc.tensor.matmul(
                            ps[:],
                            attsb[f][:, 128 * tt:128 * (tt + 1)],
                            wp[f][:, 512 * oc:512 * (oc + 1)],
                            start=(f == 0), stop=(f == 7),
                        )
                    ot = op.tile([128, 512], f32, tag="osb")
                    evict(i, ot[:], ps[:])
                    i += 1
                    nc.sync.dma_start(
                        out[128 * tt:128 * (tt + 1), 512 * oc:512 * (oc + 1)],
                        ot[:],
                    )

    nc.compile()
    _built = nc
    return nc


LAST_RESULTS = None  # BassKernelResults of the most recent kernel() call


def kernel(x: np.ndarray, W_qkv: np.ndarray, W_proj: np.ndarray) -> np.ndarray:
    global LAST_RESULTS
    from concourse import bass_utils

    nc = _build()

    x = np.ascontiguousarray(x, dtype=np.float32)
    W_qkv = np.ascontiguousarray(W_qkv, dtype=np.float32)
    W_proj = np.ascontiguousarray(W_proj, dtype=np.float32)

    # head-grouping permutation: grouped feature h*64+j <- original row j*16+h
    perm = np.arange(E).reshape(HD, NH).T.ravel()
    Wq_g = W_qkv[perm]
    Wk_g = W_qkv[E + perm] * np.float32(HD ** -0.5)  # exact: 1/8
    Wv_g = W_qkv[2 * E + perm]
    wqkvT_np = _round_fp32r(np.concatenate([Wq_g, Wk_g, Wv_g], 0).T)
    wpT_np = _round_fp32r(W_proj.T)

    in_maps = []
    for c in range(N_CORES):
        b, half = c // 2, c % 2
        xT_c = _round_fp32r(x[b, half * TPC:(half + 1) * TPC, :].T)
        in_maps.append({"xT": xT_c, "wqkvT": wqkvT_np, "wpT": wpT_np})

    res = bass_utils.run_bass_kernel_spmd(nc, in_maps, core_ids=list(range(N_CORES)))
    LAST_RESULTS = res

    out = np.empty((B, T, E), dtype=np.float32)
    for c in range(N_CORES):
        b, half = c // 2, c % 2
        out[b, half * TPC:(half + 1) * TPC, :] = res.results[c]["out"]
    return out


# revision 8
# speedup vs baseline: 1.1825x; 1.0212x over previous
"""Multi-head attention (no softmax) on 8 trn2 NeuronCores.

Reference: out = ((x @ Wqkv.T -> q,k,v per head) ; (q @ k.T * s) @ v ; concat ; @ Wproj.T)

Because there is no softmax the attention is linear:
    (q @ k.T) @ v == q @ (k.T @ v),  k.T @ v is only 64x64 per head,
so the T x T score matrices never need to exist. Per head:
    M_h = (s * k_h).T @ v_h        (64 x 64, reduced over ALL tokens of the batch)
    out += (q_h @ M_h) @ Wproj_h.T

Sharding: token-parallel. Core c owns batch b=c//2, token half c%2 (512 tokens).
M_h needs a reduction over the full batch -> two tiny 128KB AllGathers between
the two cores of each batch (pipelined, peer-add done locally on DVE),
overlapped with the second kv half and the q matmuls. A dummy 0-size collective
at kernel start absorbs the ncfw startup cost.

All matmuls run in float32r (full PE rate; fp32 is 4x slower). Inputs are
pre-rounded to fp32r on the host (matmul is then exact), intermediates are
rounded by the PSUM->SBUF eviction copies. The head-dim scale 1/8 is folded
into W_k on the host (exact, power of two).

Weights are fed pre-transposed/pre-permuted so every matmul operand has the
contraction dim on partitions with unit-stride DMAs:
  wqkvT (E, 3E): cols 0:E = q features grouped h*64+j, E:2E = k (scaled), 2E:3E = v
  wpT   (E, E):  wpT[f, o] = W_proj[o, f]
  xT_c  (E, 512) per core.

DMA triggers: Sync queue carries x/wkv/out in program order (paces the kv
phase); GpSimd queue carries wq/wp and the collective bounces, so the
collectives fire the moment their inputs are ready.
"""

import numpy as np

B, T, E = 4, 1024, 1024
NH, HD = 16, 64
N_CORES = 8
TPC = T // 2  # tokens per core = 512

_built = None


def _round_fp32r(a: np.ndarray) -> np.ndarray:
    """Round fp32 to fp32r (11 explicit mantissa bits, RNE) — matches HW."""
    u = np.ascontiguousarray(a, dtype=np.float32).view(np.uint32).astype(np.uint64)
    u = u + 0x7FF + ((u >> 12) & 1)
    u = (u & ~np.uint64(0xFFF)).astype(np.uint32)
    return u.view(np.float32).reshape(a.shape)


def _build():
    """Build + compile the 8-core SPMD Bass program once."""
    global _built
    if _built is not None:
        return _built

    import concourse.mybir as mybir
    import concourse.tile as tile
    from concourse import bacc

    f32 = mybir.dt.float32
    f32r = mybir.dt.float32r
    GROUPS = [[0, 1], [2, 3], [4, 5], [6, 7]]

    nc = bacc.Bacc("TRN2", target_bir_lowering=False, debug=False, num_devices=N_CORES)
    xT = nc.dram_tensor("xT", [E, TPC], f32r, kind="ExternalInput").ap()
    wqkvT = nc.dram_tensor("wqkvT", [E, 3 * E], f32r, kind="ExternalInput").ap()
    wpT = nc.dram_tensor("wpT", [E, E], f32r, kind="ExternalInput").ap()
    out = nc.dram_tensor("out", [TPC, E], f32, kind="ExternalOutput").ap()

    def evict(i, dst, src):
        # spread PSUM->SBUF eviction copies across DVE and ACT
        if i % 2 == 0:
            nc.vector.tensor_copy(dst, src)
        else:
            nc.scalar.copy(dst, src)

    with tile.TileContext(nc) as tc:
        with (
            tc.tile_pool(name="xp", bufs=1) as xp,
            tc.tile_pool(name="wkvp", bufs=2) as wkvp,
            tc.tile_pool(name="kvp", bufs=1) as kvp,
            tc.tile_pool(name="wqp", bufs=1) as wqp,
            tc.tile_pool(name="wpp", bufs=1) as wpp,
            tc.tile_pool(name="qp", bufs=1) as qp,
            tc.tile_pool(name="mres", bufs=1) as mres,
            tc.tile_pool(name="op", bufs=3) as op,
            tc.tile_pool(name="dram", bufs=1, space="DRAM") as dram,
            tc.tile_pool(name="psA", bufs=4, space="PSUM") as psA,
            tc.tile_pool(name="psM", bufs=2, space="PSUM") as psM,
        ):
            # dummy collective: warm up ncfw so the real gathers start fast
            dumin = dram.tile([1, 64], f32)
            dumout = dram.tile([2, 64], f32)
            nc.gpsimd.collective_compute(
                "AllGather", mybir.AluOpType.bypass, replica_groups=GROUPS,
                ins=[dumin.opt()], outs=[dumout.opt()],
            )

            # ---- input DMAs ----
            # kv fc-group order: k half 0, v half 0, k half 1, v half 1 so the
            # first half of the M blocks is ready after two groups.
            FC_ORDER = [0, 2, 1, 3]
            xsb = []
            wkv_groups = {}
            for e in range(8):
                t = xp.tile([128, TPC], f32r, tag=f"x{e}")
                nc.sync.dma_start(t[:], xT[128 * e:128 * (e + 1), :])
                xsb.append(t)
                fc = FC_ORDER[0]
                w = wkvp.tile([128, 512], f32r, tag=f"wkv{e}", name=f"wkv{fc}_{e}")
                nc.sync.dma_start(
                    w[:], wqkvT[128 * e:128 * (e + 1), E + 512 * fc:E + 512 * (fc + 1)])
                wkv_groups.setdefault(fc, []).append(w)
            for fc in FC_ORDER[1:]:
                for e in range(8):
                    w = wkvp.tile([128, 512], f32r, tag=f"wkv{e}", name=f"wkv{fc}_{e}")
                    nc.sync.dma_start(
                        w[:], wqkvT[128 * e:128 * (e + 1), E + 512 * fc:E + 512 * (fc + 1)])
                    wkv_groups.setdefault(fc, []).append(w)

            # q/proj weights on the GpSimd queue (transfers overlap kv phase)
            wq = []
            for e in range(8):
                t = wqp.tile([128, E], f32r, tag=f"wq{e}")
                nc.gpsimd.dma_start(t[:], wqkvT[128 * e:128 * (e + 1), 0:E])
                wq.append(t)
            wp = []
            for f in range(8):
                t = wpp.tile([128, E], f32r, tag=f"wp{f}")
                nc.gpsimd.dma_start(t[:], wpT[128 * f:128 * (f + 1), :])
                wp.append(t)

            kvsb = [kvp.tile([128, 2 * E], f32r, tag=f"kv{tt}", name=f"kv{tt}")
                    for tt in range(4)]
            Mbd = mres.tile([128, 1024], f32r, tag="Mbd")
            nc.gpsimd.memset(Mbd[:].bitcast(f32), 0.0)

            bout = [None, None]

            def kv_quarter(fc):
                i = 0
                for tt in range(4):
                    ps = psA.tile([128, 512], f32, tag="big")
                    for e in range(8):
                        nc.tensor.matmul(
                            ps[:],
                            xsb[e][:, 128 * tt:128 * (tt + 1)],
                            wkv_groups[fc][e][:],
                            start=(e == 0), stop=(e == 7),
                        )
                    evict(i, kvsb[tt][:, 512 * fc:512 * (fc + 1)], ps[:])
                    i += 1

            def m_half(g):
                # M blocks 4g..4g+3 from k cols [512g:512g+512], v cols
                # [E+512g : E+512g+512]; keep only diagonal 64x64 sub-blocks.
                mp = psM.tile([128, 512], f32, tag="mp", name=f"mp{g}")
                for j in range(4):
                    blk = 4 * g + j
                    for tt in range(4):
                        nc.tensor.matmul(
                            mp[:, 128 * j:128 * (j + 1)],
                            kvsb[tt][:, 128 * blk:128 * (blk + 1)],
                            kvsb[tt][:, E + 128 * blk:E + 128 * (blk + 1)],
                            start=(tt == 0), stop=(tt == 3),
                        )
                Msb = mres.tile([128, 256], f32, tag=f"Msb{g}", name=f"Msb{g}")
                for j in range(4):
                    blk = 4 * g + j
                    evict(j, Msb[0:64, 64 * j:64 * j + 64],
                          mp[0:64, 128 * j:128 * j + 64])
                    evict(j + 1, Msb[64:128, 64 * j:64 * j + 64],
                          mp[64:128, 128 * j + 64:128 * (j + 1)])
                # AllGather this half across the batch pair
                bin_ = dram.tile([128, 256], f32, name=f"bin{g}")
                bo = dram.tile([256, 256], f32, name=f"bout{g}")
                nc.gpsimd.dma_start(bin_[:], Msb[:])
                nc.gpsimd.collective_compute(
                    "AllGather", mybir.AluOpType.bypass, replica_groups=GROUPS,
                    ins=[bin_.opt()], outs=[bo.opt()],
                )
                bout[g] = bo

            def m_post(g):
                # load both ranks' partials, add straight into Mbd diagonal spots
                MrA = mres.tile([128, 256], f32, tag=f"MrA{g}", name=f"MrA{g}")
                MrB = mres.tile([128, 256], f32, tag=f"MrB{g}", name=f"MrB{g}")
                nc.gpsimd.dma_start(MrA[:], bout[g][0:128, :])
                nc.gpsimd.dma_start(MrB[:], bout[g][128:256, :])
                for j in range(4):
                    blk = 4 * g + j
                    nc.vector.tensor_add(
                        Mbd[0:64, 128 * blk:128 * blk + 64],
                        MrA[0:64, 64 * j:64 * j + 64],
                        MrB[0:64, 64 * j:64 * j + 64])
                    nc.vector.tensor_add(
                        Mbd[64:128, 128 * blk + 64:128 * (blk + 1)],
                        MrA[64:128, 64 * j:64 * j + 64],
                        MrB[64:128, 64 * j:64 * j + 64])

            # ---- kv + M + gathers, pipelined in halves ----
            kv_quarter(0)      # k cols 0:512
            kv_quarter(2)      # v cols 0:512
            m_half(0)          # M blocks 0-3 + AllGather #1 (in flight)
            kv_quarter(1)      # k cols 512:1024
            kv_quarter(3)      # v cols 512:1024
            m_half(1)          # M blocks 4-7 + AllGather #2 (in flight)

            # ---- q (feature-major qT, (1024f, 512t)), overlaps the gathers ----
            qsb = [qp.tile([128, TPC], f32r, tag=f"q{f}", name=f"q{f}")
                   for f in range(8)]
            for fq in range(8):
                ps = psA.tile([128, 512], f32, tag="big")
                for e in range(8):
                    nc.tensor.matmul(
                        ps[:],
                        wq[e][:, 128 * fq:128 * (fq + 1)],
                        xsb[e][:],
                        start=(e == 0), stop=(e == 7),
                    )
                evict(fq, qsb[fq][:], ps[:])

            m_post(0)
            m_post(1)

            # ---- att: attT_blk = Mbd_blk.T @ qT_blk (in-place into q tiles) ----
            for blk in range(8):
                ps = psA.tile([128, 512], f32, tag="big")
                nc.tensor.matmul(ps[:], Mbd[:, 128 * blk:128 * (blk + 1)],
                                 qsb[blk][:], start=True, stop=True)
                evict(blk, qsb[blk][:], ps[:])
            attsb = qsb

            # ---- out = attT.T @ wpT  ((512t, 1024o)) ----
            i = 0
            for tt in range(4):
                for oc in range(2):
                    ps = psA.tile([128, 512], f32, tag="big")
                    for f in range(8):
                        nc.tensor.matmul(
                            ps[:],
                            attsb[f][:, 128 * tt:128 * (tt + 1)],
                            wp[f][:, 512 * oc:512 * (oc + 1)],
                            start=(f == 0), stop=(f == 7),
                        )
                    ot = op.tile([128, 512], f32, tag="osb")
                    evict(i, ot[:], ps[:])
                    i += 1
                    nc.sync.dma_start(
                        out[128 * tt:128 * (tt + 1), 512 * oc:512 * (oc + 1)],
                        ot[:],
                    )

    nc.compile()
    _built = nc
    return nc


LAST_RESULTS = None  # BassKernelResults of the most recent kernel() call


def kernel(x: np.ndarray, W_qkv: np.ndarray, W_proj: np.ndarray) -> np.ndarray:
    global LAST_RESULTS
    from concourse import bass_utils

    nc = _build()

    x = np.ascontiguousarray(x, dtype=np.float32)
    W_qkv = np.ascontiguousarray(W_qkv, dtype=np.float32)
    W_proj = np.ascontiguousarray(W_proj, dtype=np.float32)

    # head-grouping permutation: grouped feature h*64+j <- original row j*16+h
    perm = np.arange(E).reshape(HD, NH).T.ravel()
    Wq_g = W_qkv[perm]
    Wk_g = W_qkv[E + perm] * np.float32(HD ** -0.5)  # exact: 1/8
    Wv_g = W_qkv[2 * E + perm]
    wqkvT_np = _round_fp32r(np.concatenate([Wq_g, Wk_g, Wv_g], 0).T)
    wpT_np = _round_fp32r(W_proj.T)

    in_maps = []
    for c in range(N_CORES):
        b, half = c // 2, c % 2
        xT_c = _round_fp32r(x[b, half * TPC:(half + 1) * TPC, :].T)
        in_maps.append({"xT": xT_c, "wqkvT": wqkvT_np, "wpT": wpT_np})

    res = bass_utils.run_bass_kernel_spmd(nc, in_maps, core_ids=list(range(N_CORES)))
    LAST_RESULTS = res

    out = np.empty((B, T, E), dtype=np.float32)
    for c in range(N_CORES):
        b, half = c // 2, c % 2
        out[b, half * TPC:(half + 1) * TPC, :] = res.results[c]["out"]
    return out


# revision 9
# speedup vs baseline: 1.3061x; 1.1045x over previous
"""Multi-head attention (no softmax) on 8 trn2 NeuronCores.

Reference: out = ((x @ Wqkv.T -> q,k,v per head) ; (q @ k.T * s) @ v ; concat ; @ Wproj.T)

Because there is no softmax the attention is linear:
    (q @ k.T) @ v == q @ (k.T @ v),  k.T @ v is only 64x64 per head,
so the T x T score matrices never need to exist. Per head:
    M_h = (s * k_h).T @ v_h        (64 x 64, reduced over ALL tokens of the batch)
    out += (q_h @ M_h) @ Wproj_h.T

Sharding: token-parallel. Core c owns batch b=c//2, token half c%2 (512 tokens).
M_h needs a reduction over the full batch -> two tiny 128KB AllGathers between
the two cores of each batch (pipelined, peer-add done locally on DVE),
overlapped with the second kv half and the q matmuls. A dummy 0-size collective
at kernel start absorbs the ncfw startup cost.

All matmuls run in float32r (full PE rate; fp32 is 4x slower). Inputs are
pre-rounded to fp32r on the host (matmul is then exact), intermediates are
rounded by the PSUM->SBUF eviction copies. The head-dim scale 1/8 is folded
into W_k on the host (exact, power of two).

Weights are fed pre-transposed/pre-permuted so every matmul operand has the
contraction dim on partitions with unit-stride DMAs:
  wqkvT (E, 3E): cols 0:E = q features grouped h*64+j, E:2E = k (scaled), 2E:3E = v
  wpT   (E, E):  wpT[f, o] = W_proj[o, f]
  xT_c  (E, 512) per core.

DMA triggers: Sync queue carries x/wkv/out in program order (paces the kv
phase); GpSimd queue carries wq/wp and the collective bounces, so the
collectives fire the moment their inputs are ready.
"""

import numpy as np

B, T, E = 4, 1024, 1024
NH, HD = 16, 64
N_CORES = 8
TPC = T // 2  # tokens per core = 512

_built = None


def _round_fp32r(a: np.ndarray) -> np.ndarray:
    """Round fp32 to fp32r (11 explicit mantissa bits, RNE) — matches HW."""
    u = np.ascontiguousarray(a, dtype=np.float32).view(np.uint32).astype(np.uint64)
    u = u + 0x7FF + ((u >> 12) & 1)
    u = (u & ~np.uint64(0xFFF)).astype(np.uint32)
    return u.view(np.float32).reshape(a.shape)


def _build():
    """Build + compile the 8-core SPMD Bass program once."""
    global _built
    if _built is not None:
        return _built

    import concourse.mybir as mybir
    import concourse.tile as tile
    from concourse import bacc

    f32 = mybir.dt.float32
    f32r = mybir.dt.float32r
    GROUPS = [[0, 1], [2, 3], [4, 5], [6, 7]]

    nc = bacc.Bacc("TRN2", target_bir_lowering=False, debug=False, num_devices=N_CORES)
    xT = nc.dram_tensor("xT", [E, TPC], f32r, kind="ExternalInput").ap()
    wqkvT = nc.dram_tensor("wqkvT", [E, 3 * E], f32r, kind="ExternalInput").ap()
    wpT = nc.dram_tensor("wpT", [E, E], f32r, kind="ExternalInput").ap()
    out = nc.dram_tensor("out", [TPC, E], f32, kind="ExternalOutput").ap()

    def evict(i, dst, src):
        # spread PSUM->SBUF eviction copies across DVE and ACT
        if i % 2 == 0:
            nc.vector.tensor_copy(dst, src)
        else:
            nc.scalar.copy(dst, src)

    with tile.TileContext(nc) as tc:
        with (
            tc.tile_pool(name="xp", bufs=1) as xp,
            tc.tile_pool(name="wkvp", bufs=3) as wkvp,
            tc.tile_pool(name="kvp", bufs=1) as kvp,
            tc.tile_pool(name="wqp", bufs=1) as wqp,
            tc.tile_pool(name="wpp", bufs=1) as wpp,
            tc.tile_pool(name="qp", bufs=1) as qp,
            tc.tile_pool(name="mres", bufs=1) as mres,
            tc.tile_pool(name="op", bufs=3) as op,
            tc.tile_pool(name="dram", bufs=1, space="DRAM") as dram,
            tc.tile_pool(name="psA", bufs=4, space="PSUM") as psA,
            tc.tile_pool(name="psM", bufs=2, space="PSUM") as psM,
        ):
            # ---- input DMAs ----
            # kv fc-group order: k half 0, v half 0, k half 1, v half 1 so the
            # first half of the M blocks is ready after two groups.
            FC_ORDER = [0, 2, 1, 3]
            xsb = []
            wkv_groups = {}
            for e in range(8):
                t = xp.tile([128, TPC], f32r, tag=f"x{e}")
                nc.sync.dma_start(t[:], xT[128 * e:128 * (e + 1), :])
                xsb.append(t)
                fc = FC_ORDER[0]
                w = wkvp.tile([128, 512], f32r, tag=f"wkv{e}", name=f"wkv{fc}_{e}")
                nc.sync.dma_start(
                    w[:], wqkvT[128 * e:128 * (e + 1), E + 512 * fc:E + 512 * (fc + 1)])
                wkv_groups.setdefault(fc, []).append(w)
            for fc in FC_ORDER[1:]:
                for e in range(8):
                    w = wkvp.tile([128, 512], f32r, tag=f"wkv{e}", name=f"wkv{fc}_{e}")
                    nc.sync.dma_start(
                        w[:], wqkvT[128 * e:128 * (e + 1), E + 512 * fc:E + 512 * (fc + 1)])
                    wkv_groups.setdefault(fc, []).append(w)

            # q/proj weights on the Sync queue AFTER the kv weights so their
            # transfers don't steal HBM bandwidth from the critical kv stream
            wq = []
            for e in range(8):
                t = wqp.tile([128, E], f32r, tag=f"wq{e}")
                nc.sync.dma_start(t[:], wqkvT[128 * e:128 * (e + 1), 0:E])
                wq.append(t)
            wp = []
            for f in range(8):
                t = wpp.tile([128, E], f32r, tag=f"wp{f}")
                nc.sync.dma_start(t[:], wpT[128 * f:128 * (f + 1), :])
                wp.append(t)

            kvsb = [kvp.tile([128, 2 * E], f32r, tag=f"kv{tt}", name=f"kv{tt}")
                    for tt in range(4)]
            Mbd = mres.tile([128, 1024], f32r, tag="Mbd")
            nc.gpsimd.memset(Mbd[:].bitcast(f32), 0.0)

            bout = [None, None]

            def kv_quarter(fc):
                i = 0
                for tt in range(4):
                    ps = psA.tile([128, 512], f32, tag="big")
                    for e in range(8):
                        nc.tensor.matmul(
                            ps[:],
                            xsb[e][:, 128 * tt:128 * (tt + 1)],
                            wkv_groups[fc][e][:],
                            start=(e == 0), stop=(e == 7),
                        )
                    evict(i, kvsb[tt][:, 512 * fc:512 * (fc + 1)], ps[:])
                    i += 1

            def m_half(g):
                # M blocks 4g..4g+3 from k cols [512g:512g+512], v cols
                # [E+512g : E+512g+512]; keep only diagonal 64x64 sub-blocks.
                mp = psM.tile([128, 512], f32, tag="mp", name=f"mp{g}")
                for j in range(4):
                    blk = 4 * g + j
                    for tt in range(4):
                        nc.tensor.matmul(
                            mp[:, 128 * j:128 * (j + 1)],
                            kvsb[tt][:, 128 * blk:128 * (blk + 1)],
                            kvsb[tt][:, E + 128 * blk:E + 128 * (blk + 1)],
                            start=(tt == 0), stop=(tt == 3),
                        )
                Msb = mres.tile([128, 256], f32, tag=f"Msb{g}", name=f"Msb{g}")
                for j in range(4):
                    blk = 4 * g + j
                    evict(j, Msb[0:64, 64 * j:64 * j + 64],
                          mp[0:64, 128 * j:128 * j + 64])
                    evict(j + 1, Msb[64:128, 64 * j:64 * j + 64],
                          mp[64:128, 128 * j + 64:128 * (j + 1)])
                # AllGather this half across the batch pair
                bin_ = dram.tile([128, 256], f32, name=f"bin{g}")
                bo = dram.tile([256, 256], f32, name=f"bout{g}")
                nc.gpsimd.dma_start(bin_[:], Msb[:])
                nc.gpsimd.collective_compute(
                    "AllGather", mybir.AluOpType.bypass, replica_groups=GROUPS,
                    ins=[bin_.opt()], outs=[bo.opt()],
                )
                MrA = mres.tile([128, 256], f32, tag=f"MrA{g}", name=f"MrA{g}")
                MrB = mres.tile([128, 256], f32, tag=f"MrB{g}", name=f"MrB{g}")
                nc.gpsimd.dma_start(MrA[:], bo[0:128, :])
                nc.gpsimd.dma_start(MrB[:], bo[128:256, :])
                bout[g] = (MrA, MrB)

            def m_post(g):
                # add both ranks' partials straight into Mbd diagonal spots
                MrA, MrB = bout[g]
                for j in range(4):
                    blk = 4 * g + j
                    nc.vector.tensor_add(
                        Mbd[0:64, 128 * blk:128 * blk + 64],
                        MrA[0:64, 64 * j:64 * j + 64],
                        MrB[0:64, 64 * j:64 * j + 64])
                    nc.vector.tensor_add(
                        Mbd[64:128, 128 * blk + 64:128 * (blk + 1)],
                        MrA[64:128, 64 * j:64 * j + 64],
                        MrB[64:128, 64 * j:64 * j + 64])

            # ---- kv + M + gathers, pipelined in halves ----
            kv_quarter(0)      # k cols 0:512
            kv_quarter(2)      # v cols 0:512
            m_half(0)          # M blocks 0-3 + AllGather #1 (in flight)
            kv_quarter(1)      # k cols 512:1024
            kv_quarter(3)      # v cols 512:1024
            m_half(1)          # M blocks 4-7 + AllGather #2 (in flight)

            # ---- q (feature-major qT, (1024f, 512t)), overlaps the gathers ----
            qsb = [qp.tile([128, TPC], f32r, tag=f"q{f}", name=f"q{f}")
                   for f in range(8)]
            for fq in range(8):
                ps = psA.tile([128, 512], f32, tag="big")
                for e in range(8):
                    nc.tensor.matmul(
                        ps[:],
                        wq[e][:, 128 * fq:128 * (fq + 1)],
                        xsb[e][:],
                        start=(e == 0), stop=(e == 7),
                    )
                evict(fq, qsb[fq][:], ps[:])

            m_post(0)
            m_post(1)

            # ---- att: attT_blk = Mbd_blk.T @ qT_blk (in-place into q tiles) ----
            for blk in range(8):
                ps = psA.tile([128, 512], f32, tag="big")
                nc.tensor.matmul(ps[:], Mbd[:, 128 * blk:128 * (blk + 1)],
                                 qsb[blk][:], start=True, stop=True)
                evict(blk, qsb[blk][:], ps[:])
            attsb = qsb

            # ---- out = attT.T @ wpT  ((512t, 1024o)) ----
            i = 0
            for tt in range(4):
                for oc in range(2):
                    ps = psA.tile([128, 512], f32, tag="big")
                    for f in range(8):
                        nc.tensor.matmul(
                            ps[:],
                            attsb[f][:, 128 * tt:128 * (tt + 1)],
                            wp[f][:, 512 * oc:512 * (oc + 1)],
                            start=(f == 0), stop=(f == 7),
                        )
                    ot = op.tile([128, 512], f32, tag="osb")
                    evict(i, ot[:], ps[:])
                    i += 1
                    nc.sync.dma_start(
                        out[128 * tt:128 * (tt + 1), 512 * oc:512 * (oc + 1)],
                        ot[:],
                    )

    nc.compile()
    _built = nc
    return nc


LAST_RESULTS = None  # BassKernelResults of the most recent kernel() call


def kernel(x: np.ndarray, W_qkv: np.ndarray, W_proj: np.ndarray) -> np.ndarray:
    global LAST_RESULTS
    from concourse import bass_utils

    nc = _build()

    x = np.ascontiguousarray(x, dtype=np.float32)
    W_qkv = np.ascontiguousarray(W_qkv, dtype=np.float32)
    W_proj = np.ascontiguousarray(W_proj, dtype=np.float32)

    # head-grouping permutation: grouped feature h*64+j <- original row j*16+h
    perm = np.arange(E).reshape(HD, NH).T.ravel()
    Wq_g = W_qkv[perm]
    Wk_g = W_qkv[E + perm] * np.float32(HD ** -0.5)  # exact: 1/8
    Wv_g = W_qkv[2 * E + perm]
    wqkvT_np = _round_fp32r(np.concatenate([Wq_g, Wk_g, Wv_g], 0).T)
    wpT_np = _round_fp32r(W_proj.T)

    in_maps = []
    for c in range(N_CORES):
        b, half = c // 2, c % 2
        xT_c = _round_fp32r(x[b, half * TPC:(half + 1) * TPC, :].T)
        in_maps.append({"xT": xT_c, "wqkvT": wqkvT_np, "wpT": wpT_np})

    res = bass_utils.run_bass_kernel_spmd(nc, in_maps, core_ids=list(range(N_CORES)))
    LAST_RESULTS = res

    out = np.empty((B, T, E), dtype=np.float32)
    for c in range(N_CORES):
        b, half = c // 2, c % 2
        out[b, half * TPC:(half + 1) * TPC, :] = res.results[c]["out"]
    return out
